# revision 11
# baseline (speedup 1.0000x reference)
"""Trainium2 Bass kernel for nn_AdaptiveCoFusion (B=8, L=128, R=49, D=768).

Pure data parallel: one batch element per NeuronCore (8 cores), weights
replicated, host-packed to bf16 in SBUF layout (one contiguous DMA per
pack).

Per-core structure (v2 — "natural orientation"):
  Big projections run with the TRANSPOSED ACTIVATION as the stationary
  operand and fused weight packs as the moving operand (N=512 chunks):
    GA: lhsT=txt^T,  rhs=[Wt1|Wt2|Wout_t]  -> y1, y3, out(partial)
    GV: lhsT=vis^T,  rhs=Wi1               -> yv
    GB: lhsT=att_img^T, rhs=[Wi2|Wgi]      -> y2, new_img
    GC: lhsT=att_text^T, rhs=Wgt           -> new_txt
    GD: lhsT=mm^T,   rhs=Wrv               -> rv
    GE: lhsT=res^T,  rhs=Wout_m            -> out(final accumulate)
  A post-compile BIR pass deletes sync-free InstLdweights that reload
  the identical stationary operand, so each group pays ~1 weight load
  per K-chunk instead of one per matmul.
  Additive-attention score vectors are computed as fused DVE
  multiply-reduce against partition-broadcast vectors (loaded with a
  stride-0 broadcast DMA); u[i]+v[j] materializes as one rank-1 matmul
  (ones (x) v_row) with u riding the ScalarE Exp bias input. Gates are
  per-partition columns consumed directly by tensor_scalar ops.
  Sigmoid = 0.5*tanh(0.5x)+0.5 keeps ScalarE in one table set. The
  FiltrationGate projections fold on host: (txt@Wft)@wfg_t ==
  txt@(Wft@wfg_t), so Wft/Wfm are never shipped.
"""

import os
import numpy as np
import ml_dtypes

B, L, R, D = 8, 128, 49, 768
KC = D // 128  # 6
BF_NP = ml_dtypes.bfloat16

LAST = None  # BassKernelResults of the most recent run (for test harness)
LDW_DROPPED = 0
_CACHE = {}


def _pack_w(w):
    # (768, ncols) -> (128, KC*ncols): [p, kc*ncols + n] = w[kc*128 + p, n]
    ncols = w.shape[1]
    return np.ascontiguousarray(
        w.reshape(KC, 128, ncols).transpose(1, 0, 2).reshape(128, KC * ncols)
    ).astype(BF_NP)


def _dedup_ldweights(nc, mybir):
    """Drop sync-free InstLdweights that reload the PE stationary operand
    already resident from the previous load. Conservative: any intervening
    PE load (incl. transposes) resets the tracked state; lds carrying
    semaphore waits/updates are never dropped."""
    dropped = 0
    for blk in nc.m.functions[0].blocks:
        last_w = None
        new = []
        for i in blk.instructions:
            if getattr(i, "engine", None) == mybir.EngineType.PE and \
                    isinstance(i, mybir.InstLdweights):
                w = str(i.ins[0])
                si = i.sync_info
                clean = si is None or (not si.on_wait and not si.on_update)
                if w == last_w and clean:
                    dropped += 1
                    continue
                last_w = w
            new.append(i)
        blk.instructions = new
    return dropped


def _build(bias_flags):
    from contextlib import ExitStack
    import concourse.bass as bass
    import concourse.tile as tile
    from concourse import bacc, mybir
    from concourse.alu_op_type import AluOpType
    global LDW_DROPPED

    ga_bias, gb_bias, gc_bias, gd_bias, ge_bias = bias_flags

    F32 = mybir.dt.float32
    BF = mybir.dt.bfloat16
    AF = mybir.ActivationFunctionType
    MUL, ADD = AluOpType.mult, AluOpType.add

    nc = bacc.Bacc("TRN2", target_bir_lowering=False, debug=False,
                   enable_asserts=False)

    txt_d = nc.dram_tensor("txt", [L, D], BF, kind="ExternalInput").ap()
    vis_d = nc.dram_tensor("vis", [R, D], BF, kind="ExternalInput").ap()
    wA_d = nc.dram_tensor("wA", [128, KC * 2304], BF, kind="ExternalInput").ap()
    wV_d = nc.dram_tensor("wV", [128, KC * D], BF, kind="ExternalInput").ap()
    wB_d = nc.dram_tensor("wB", [128, KC * 1536], BF, kind="ExternalInput").ap()
    wC_d = nc.dram_tensor("wC", [128, KC * D], BF, kind="ExternalInput").ap()
    wD_d = nc.dram_tensor("wD", [128, KC * D], BF, kind="ExternalInput").ap()
    wE_d = nc.dram_tensor("wE", [128, KC * D], BF, kind="ExternalInput").ap()
    vbc_d = nc.dram_tensor("vbc", [1, 8 * D], BF, kind="ExternalInput").ap()
    id_d = nc.dram_tensor("identd", [128, 128], BF, kind="ExternalInput").ap()
    scal_d = nc.dram_tensor("scal", [128, 4], F32, kind="ExternalInput").ap()
    brows_d = nc.dram_tensor("brows", [1, 5376], BF, kind="ExternalInput").ap()
    out_d = nc.dram_tensor("out", [L, D], F32, kind="ExternalOutput").ap()

    # vbc blocks: 0 wa1_t, 1 wa1_i, 2 wa2_i, 3 wa2_t, 4 wg_i, 5 wg_t, 6 c_t, 7 c_m
    VB = lambda i: slice(i * D, (i + 1) * D)

    with tile.TileContext(nc) as tc, ExitStack() as ctx:
        const = ctx.enter_context(tc.tile_pool(name="const", bufs=1))
        wpool = ctx.enter_context(tc.tile_pool(name="wpool", bufs=1))
        acts = ctx.enter_context(tc.tile_pool(name="acts", bufs=1))
        tmp = ctx.enter_context(tc.tile_pool(name="tmp", bufs=2))
        psum = ctx.enter_context(tc.tile_pool(name="psum", bufs=1, space="PSUM"))
        psm = ctx.enter_context(tc.tile_pool(name="psm", bufs=3, space="PSUM"))

        # ---- DMAs. sync ring (HWDGE): txt + the big packs in consumption
        # order; gpsimd ring (SWDGE): vis/ident/wV/wC + broadcast + tiny.
        txt_bf = const.tile([L, D], BF, tag="txt")
        nc.sync.dma_start(out=txt_bf, in_=txt_d)
        wA_sb = wpool.tile([128, KC * 2304], BF, tag="wA")
        h = KC // 2 * 2304
        nc.sync.dma_start(out=wA_sb[:, 0:h], in_=wA_d[:, 0:h])
        nc.sync.dma_start(out=wA_sb[:, h:], in_=wA_d[:, h:])
        wB_sb = wpool.tile([128, KC * 1536], BF, tag="wB")
        h2 = KC // 2 * 1536
        nc.sync.dma_start(out=wB_sb[:, 0:h2], in_=wB_d[:, 0:h2])
        nc.sync.dma_start(out=wB_sb[:, h2:], in_=wB_d[:, h2:])
        wD_sb = wpool.tile([128, KC * D], BF, tag="wD")
        nc.sync.dma_start(out=wD_sb, in_=wD_d)
        wE_sb = wpool.tile([128, KC * D], BF, tag="wE")
        nc.sync.dma_start(out=wE_sb, in_=wE_d)

        vis_bf = const.tile([R, D], BF, tag="vis")
        nc.gpsimd.dma_start(out=vis_bf, in_=vis_d)
        ident = const.tile([128, 128], BF, tag="ident")
        nc.gpsimd.dma_start(out=ident, in_=id_d)
        wV_sb = wpool.tile([128, KC * D], BF, tag="wV")
        nc.gpsimd.dma_start(out=wV_sb, in_=wV_d)
        wC_sb = wpool.tile([128, KC * D], BF, tag="wC")
        nc.gpsimd.dma_start(out=wC_sb, in_=wC_d)

        vbc_sb = const.tile([128, 8 * D], BF, tag="vbc")
        vbc_bc = bass.AP(tensor=vbc_d.tensor, offset=vbc_d.offset,
                         ap=[[0, 128]] + [list(p) for p in vbc_d.ap[1:]])
        nc.gpsimd.dma_start(out=vbc_sb, in_=vbc_bc)
        scal_sb = const.tile([128, 4], F32, tag="scal")
        nc.gpsimd.dma_start(out=scal_sb, in_=scal_d)
        brows_sb = const.tile([1, 5376], BF, tag="brows")
        nc.gpsimd.dma_start(out=brows_sb, in_=brows_d)

        ones_row = const.tile([1, 128], BF, tag="ones")
        nc.vector.memset(ones_row, 1.0)

        # ---- transposes: txt^T, vis^T
        txtT = acts.tile([128, KC * 128], BF, tag="txtT")
        for kc in range(KC):
            ps = psm.tile([128, 128], BF, tag="sm")
            nc.tensor.transpose(ps, txt_bf[:, kc * 128:(kc + 1) * 128], ident)
            nc.vector.tensor_copy(txtT[:, kc * 128:(kc + 1) * 128], ps)
        visT = acts.tile([128, KC * R], BF, tag="visT")
        for kc in range(KC):
            ps = psm.tile([128, 128], BF, tag="sm")
            nc.tensor.transpose(ps[:, 0:R], vis_bf[:, kc * 128:(kc + 1) * 128],
                                ident[0:R, 0:R])
            nc.vector.tensor_copy(visT[:, kc * R:(kc + 1) * R], ps[:, 0:R])

        def group(ps_chunks, lhsT_src, lhsT_w, w_sb, wcols, bias_row):
            """One fused matmul group: for each kc, one stationary load of
            lhsT_src[:, kc chunk] reused across all moving chunks.
            ps_chunks: list of (psum_ap, col0, col1, opens, closes)."""
            for kc in range(KC):
                base = kc * wcols
                for ci, (pap, c0, c1, opens, closes) in enumerate(ps_chunks):
                    nc.tensor.matmul(
                        pap,
                        lhsT=lhsT_src[:, kc * lhsT_w:(kc + 1) * lhsT_w],
                        rhs=w_sb[:, base + c0: base + c1],
                        start=(opens and kc == 0),
                        stop=(closes and kc == KC - 1 and bias_row is None),
                    )
            if bias_row is not None:
                for ci, (pap, c0, c1, opens, closes) in enumerate(ps_chunks):
                    nc.tensor.matmul(
                        pap, lhsT=ones_row,
                        rhs=brows_sb[:, bias_row + c0: bias_row + c1],
                        start=False, stop=closes)

        def ttr(dst_col, in0, in1, parts=128):
            scr = tmp.tile([128, D], BF, tag="scr")
            nc.vector.tensor_mul(scr[0:parts], in0, in1)
            nc.vector.reduce_sum(dst_col, scr[0:parts],
                                 axis=mybir.AxisListType.X)

        def tanh_chunks(dst, ps, total, bias=0.0):
            for c0 in range(0, total, 512):
                c1 = min(c0 + 512, total)
                nc.scalar.activation(out=dst[:, c0:c1], in_=ps[:, c0:c1],
                                     func=AF.Tanh, bias=bias)

        # ---- GA: y1|y3|out_partial from txt^T
        out_ps = psum.tile([128, D], F32, tag="out")
        ya_ps = psum.tile([128, 1536], F32, tag="big")
        group([(ya_ps[:, 0:512], 0, 512, True, True),
               (ya_ps[:, 512:1024], 512, 1024, True, True),
               (ya_ps[:, 1024:1536], 1024, 1536, True, True),
               (out_ps[:, 0:512], 1536, 2048, True, False),
               (out_ps[:, 512:768], 2048, 2304, True, False)],
              txtT, 128, wA_sb, 2304, 0 if ga_bias else None)
        y13 = acts.tile([128, 1536], BF, tag="y13")
        tanh_chunks(y13, ya_ps, 1536)

        u1c = acts.tile([128, 1], F32, tag="u1c")
        ttr(u1c, y13[:, 0:D], vbc_sb[:, VB(0)])
        v2c = acts.tile([128, 1], F32, tag="v2c")
        ttr(v2c, y13[:, D:2 * D], vbc_sb[:, VB(3)])

        # ---- GV: yv from vis^T
        gv_ps = psum.tile([128, 1536], F32, tag="big")
        group([(gv_ps[0:R, 0:512], 0, 512, True, True),
               (gv_ps[0:R, 512:768], 512, 768, True, True)],
              visT, R, wV_sb, D, None)
        yv = acts.tile([R, D], BF, tag="yv")
        for c0, c1 in ((0, 512), (512, 768)):
            nc.scalar.activation(out=yv[:, c0:c1], in_=gv_ps[0:R, c0:c1],
                                 func=AF.Tanh)
        v1c = acts.tile([R, 1], F32, tag="v1c")
        ttr(v1c, yv, vbc_sb[0:R, VB(1)], parts=R)

        # ---- scores1 = exp(v1[r] + u1[l] + ba1); softmax over r; probs1^T
        v1cb = acts.tile([R, 1], BF, tag="v1cb")
        nc.vector.tensor_copy(v1cb, v1c)
        ps_v1r = psm.tile([1, 128], BF, tag="sm")
        nc.tensor.transpose(ps_v1r[:, 0:R], v1cb, ident[0:R, 0:R])
        v1r = acts.tile([1, R], BF, tag="v1r")
        nc.vector.tensor_copy(v1r, ps_v1r[:, 0:R])
        u1b = acts.tile([128, 1], F32, tag="u1b")
        nc.vector.tensor_scalar_add(u1b, u1c, scal_sb[:, 0:1])

        s1_ps = psm.tile([128, R], F32, tag="sm")
        nc.tensor.matmul(s1_ps, lhsT=ones_row, rhs=v1r, start=True, stop=True)
        probs1 = acts.tile([128, R], F32, tag="p1")
        nc.scalar.activation(out=probs1, in_=s1_ps, func=AF.Exp, bias=u1b)
        rs1 = acts.tile([128, 1], F32, tag="rs1")
        nc.vector.reduce_sum(rs1, probs1, axis=mybir.AxisListType.X)
        rr1 = acts.tile([128, 1], F32, tag="rr1")
        nc.vector.reciprocal(rr1, rs1)
        p1b = acts.tile([128, R], BF, tag="p1b")
        nc.vector.tensor_scalar_mul(p1b, probs1, rr1)
        ps_p1t = psm.tile([R, 128], BF, tag="sm")
        nc.tensor.transpose(ps_p1t, p1b, ident)
        p1T = acts.tile([R, 128], BF, tag="p1T")
        nc.vector.tensor_copy(p1T, ps_p1t)

        # ---- att_img^T via PV (output lands transposed)
        aimgT = acts.tile([128, KC * 128], BF, tag="aimgT")
        for mc in range(KC):
            ps = psm.tile([128, 128], F32, tag="sm")
            nc.tensor.matmul(ps, lhsT=vis_bf[:, mc * 128:(mc + 1) * 128],
                             rhs=p1T, start=True, stop=True)
            nc.vector.tensor_copy(aimgT[:, mc * 128:(mc + 1) * 128], ps)

        # ---- GB: y2|new_img from att_img^T
        yb_ps = psum.tile([128, 1536], F32, tag="big")
        group([(yb_ps[:, 0:512], 0, 512, True, True),
               (yb_ps[:, 512:1024], 512, 1024, True, True),
               (yb_ps[:, 1024:1536], 1024, 1536, True, True)],
              aimgT, 128, wB_sb, 1536, 2304 if gb_bias else None)
        y2ni = acts.tile([128, 1536], BF, tag="y2ni")
        tanh_chunks(y2ni, yb_ps, 1536)
        ni = y2ni[:, D:2 * D]

        u2c = acts.tile([128, 1], F32, tag="u2c")
        ttr(u2c, y2ni[:, 0:D], vbc_sb[:, VB(2)])
        u2b = acts.tile([128, 1], F32, tag="u2b")
        nc.vector.tensor_scalar_add(u2b, u2c, scal_sb[:, 1:2])

        # ---- scores2 = exp(v2[j] + u2[i] + ba2); softmax over j; probs2^T
        v2cb = acts.tile([128, 1], BF, tag="v2cb")
        nc.vector.tensor_copy(v2cb, v2c)
        ps_v2r = psm.tile([1, 128], BF, tag="sm")
        nc.tensor.transpose(ps_v2r, v2cb, ident)
        v2r = acts.tile([1, 128], BF, tag="v2r")
        nc.vector.tensor_copy(v2r, ps_v2r)

        s2_ps = psm.tile([128, 128], F32, tag="sm")
        nc.tensor.matmul(s2_ps, lhsT=ones_row, rhs=v2r, start=True, stop=True)
        probs2 = acts.tile([128, 128], F32, tag="p2")
        nc.scalar.activation(out=probs2, in_=s2_ps, func=AF.Exp, bias=u2b)
        rs2 = acts.tile([128, 1], F32, tag="rs2")
        nc.vector.reduce_sum(rs2, probs2, axis=mybir.AxisListType.X)
        rr2 = acts.tile([128, 1], F32, tag="rr2")
        nc.vector.reciprocal(rr2, rs2)
        p2b = acts.tile([128, 128], BF, tag="p2b")
        nc.vector.tensor_scalar_mul(p2b, probs2, rr2)
        ps_p2t = psm.tile([128, 128], BF, tag="sm")
        nc.tensor.transpose(ps_p2t, p2b, ident)
        p2T = acts.tile([128, 128], BF, tag="p2T")
        nc.vector.tensor_copy(p2T, ps_p2t)

        # ---- att_text^T via PV2
        atxtT = acts.tile([128, KC * 128], BF, tag="atxtT")
        for mc in range(KC):
            ps = psm.tile([128, 128], F32, tag="sm")
            nc.tensor.matmul(ps, lhsT=txt_bf[:, mc * 128:(mc + 1) * 128],
                             rhs=p2T, start=True, stop=True)
            nc.vector.tensor_copy(atxtT[:, mc * 128:(mc + 1) * 128], ps)

        # ---- GC: new_txt from att_text^T
        gc_ps = psum.tile([128, 1536], F32, tag="big")
        group([(gc_ps[:, 0:512], 0, 512, True, True),
               (gc_ps[:, 512:768], 512, 768, True, True)],
              atxtT, 128, wC_sb, D, 3840 if gc_bias else None)
        nt = acts.tile([128, D], BF, tag="nt")
        tanh_chunks(nt, gc_ps, D)

        # ---- GMF gate (per-partition column) + multimodal
        zgi = acts.tile([128, 1], F32, tag="zgi")
        ttr(zgi, ni, vbc_sb[:, VB(4)])
        zgt = acts.tile([128, 1], F32, tag="zgt")
        ttr(zgt, nt, vbc_sb[:, VB(5)])
        zg = acts.tile([128, 1], F32, tag="zg")
        nc.vector.tensor_add(zg, zgi, zgt)
        tg = acts.tile([128, 1], F32, tag="tg")
        nc.scalar.activation(out=tg, in_=zg, func=AF.Tanh, scale=0.5)
        g_col = acts.tile([128, 1], F32, tag="gcol")
        nc.vector.tensor_scalar(g_col, tg, 0.5, 0.5, MUL, ADD)

        mm_nat = acts.tile([128, D], BF, tag="mmn")
        dmm = tmp.tile([128, D], BF, tag="dmm")
        nc.vector.tensor_sub(dmm, ni, nt)
        dms = tmp.tile([128, D], BF, tag="dms")
        nc.vector.tensor_scalar_mul(dms, dmm, g_col)
        nc.vector.tensor_add(mm_nat, nt, dms)

        # ---- mm^T for GD
        mmT = acts.tile([128, KC * 128], BF, tag="mmT")
        for kc in range(KC):
            ps = psm.tile([128, 128], BF, tag="sm")
            nc.tensor.transpose(ps, mm_nat[:, kc * 128:(kc + 1) * 128], ident)
            nc.vector.tensor_copy(mmT[:, kc * 128:(kc + 1) * 128], ps)

        # ---- FiltrationGate column (host-folded c_t, c_m)
        zf1 = acts.tile([128, 1], F32, tag="zf1")
        ttr(zf1, txt_bf, vbc_sb[:, VB(6)])
        zf2 = acts.tile([128, 1], F32, tag="zf2")
        ttr(zf2, mm_nat, vbc_sb[:, VB(7)])
        zf = acts.tile([128, 1], F32, tag="zf")
        nc.vector.tensor_add(zf, zf1, zf2)
        tf = acts.tile([128, 1], F32, tag="tf")
        nc.scalar.activation(out=tf, in_=zf, func=AF.Tanh, scale=0.5,
                             bias=scal_sb[:, 2:3])
        f_col = acts.tile([128, 1], F32, tag="fcol")
        nc.vector.tensor_scalar(f_col, tf, 0.5, 0.5, MUL, ADD)

        # ---- GD: rv from mm^T ; reserved = f * rv
        gd_ps = psum.tile([128, 1536], F32, tag="big")
        group([(gd_ps[:, 0:512], 0, 512, True, True),
               (gd_ps[:, 512:768], 512, 768, True, True)],
              mmT, 128, wD_sb, D, 4608 if gd_bias else None)
        rv = acts.tile([128, D], BF, tag="rv")
        tanh_chunks(rv, gd_ps, D)
        res = acts.tile([128, D], BF, tag="res")
        nc.vector.tensor_scalar_mul(res, rv, f_col)

        # ---- res^T for GE
        resT = acts.tile([128, KC * 128], BF, tag="resT")
        for kc in range(KC):
            ps = psm.tile([128, 128], BF, tag="sm")
            nc.tensor.transpose(ps, res[:, kc * 128:(kc + 1) * 128], ident)
            nc.vector.tensor_copy(resT[:, kc * 128:(kc + 1) * 128], ps)

        # ---- GE: accumulate res@Wout_m into out_ps (+ bout), write out
        group([(out_ps[:, 0:512], 0, 512, False, True),
               (out_ps[:, 512:768], 512, 768, False, True)],
              resT, 128, wE_sb, D, None if not ge_bias else 1536)
        out_sb = acts.tile([L, D], F32, tag="outsb")
        for c0, c1 in ((0, 512), (512, 768)):
            nc.vector.tensor_copy(out_sb[:, c0:c1], out_ps[:, c0:c1])
            nc.sync.dma_start(out=out_d[:, c0:c1], in_=out_sb[:, c0:c1])

    nc.compile()
    LDW_DROPPED = _dedup_ldweights(nc, mybir)
    return nc


def _inputs_pack(inp):
    f32 = np.float32
    g = lambda k: np.asarray(inp[k], dtype=f32)

    wA = _pack_w(np.concatenate([g("Wt1"), g("Wt2"), g("Wout_t")], axis=1))
    wV = _pack_w(g("Wi1"))
    wB = _pack_w(np.concatenate([g("Wi2"), g("Wgi")], axis=1))
    wC = _pack_w(g("Wgt"))
    wD = _pack_w(g("Wrv"))
    wE = _pack_w(g("Wout_m"))

    c_t = g("Wft").astype(np.float64) @ g("wfg_t").astype(np.float64)
    c_m = g("Wfm").astype(np.float64) @ g("wfg_m").astype(np.float64)
    s_fh = 0.5 * (float(g("bfm").astype(np.float64) @ g("wfg_m").astype(np.float64))
                  + float(g("bfg")))

    vbc = np.concatenate([g("wa1_t"), g("wa1_i"), g("wa2_i"), g("wa2_t"),
                          g("wg_i"), g("wg_t"),
                          c_t.astype(f32), c_m.astype(f32)]).reshape(1, 8 * D)
    vbc = vbc.astype(BF_NP)

    scal = np.zeros((128, 4), f32)
    scal[:, 0] = float(g("ba1"))
    scal[:, 1] = float(g("ba2"))
    scal[:, 2] = s_fh

    brows = np.zeros((1, 5376), f32)
    brows[0, 0:768] = g("bt1")
    brows[0, 1536:2304] = g("bout")
    brows[0, 2304:3072] = g("bi2")
    brows[0, 3072:3840] = g("bgi")
    brows[0, 3840:4608] = g("bgt")
    brows[0, 4608:5376] = g("brv")
    bias_flags = (
        bool(np.any(g("bt1")) or np.any(g("bout"))),  # ga (bt1; bout w/ GE)
        bool(np.any(g("bi2")) or np.any(g("bgi"))),   # gb
        bool(np.any(g("bgt"))),                        # gc
        bool(np.any(g("brv"))),                        # gd
        False,                                         # ge (bout rides GA)
    )
    # bout rides GA's bias row range [1536:2304] only if ga_bias; if only
    # bout is nonzero, GE emits it from brows[1536:2304] via ge flag.
    brows = brows.astype(BF_NP)

    ident = np.eye(128, dtype=BF_NP)

    shared = dict(wA=wA, wV=wV, wB=wB, wC=wC, wD=wD, wE=wE, vbc=vbc,
                  identd=ident, scal=scal, brows=brows)

    txt = g("txt_hidden").astype(BF_NP)
    vis = g("vis_hidden").astype(BF_NP)
    in_maps = []
    for c in range(B):
        m = dict(shared)
        m["txt"] = np.ascontiguousarray(txt[c])
        m["vis"] = np.ascontiguousarray(vis[c])
        in_maps.append(m)
    return in_maps, bias_flags


def kernel(**inputs):
    global LAST
    from concourse import bass_utils

    in_maps, bias_flags = _inputs_pack(inputs)
    key = ("v2", bias_flags)
    nc = _CACHE.get(key)
    if nc is None:
        nc = _build(bias_flags)
        _CACHE[key] = nc

    res = bass_utils.run_bass_kernel_spmd(
        nc, in_maps, core_ids=list(range(B)),
        trace=bool(os.environ.get("KERNEL_TRACE")),
    )
    LAST = res
    out = np.stack([np.asarray(res.results[c]["out"]) for c in range(B)], axis=0)
    return out.astype(np.float32)


# revision 12
# speedup vs baseline: 1.3857x; 1.3857x over previous
"""Trainium2 Bass kernel for nn_AdaptiveCoFusion (B=8, L=128, R=49, D=768).

Pure data parallel: one batch element per NeuronCore (8 cores), weights
replicated, host-packed to bf16 in SBUF layout (one contiguous DMA per
pack).

Per-core structure (v2 — "natural orientation"):
  Big projections run with the TRANSPOSED ACTIVATION as the stationary
  operand and fused weight packs as the moving operand (N=512 chunks):
    GA: lhsT=txt^T,  rhs=[Wt1|Wt2|Wout_t]  -> y1, y3, out(partial)
    GV: lhsT=vis^T,  rhs=Wi1               -> yv
    GB: lhsT=att_img^T, rhs=[Wi2|Wgi]      -> y2, new_img
    GC: lhsT=att_text^T, rhs=Wgt           -> new_txt
    GD: lhsT=mm^T,   rhs=Wrv               -> rv
    GE: lhsT=res^T,  rhs=Wout_m            -> out(final accumulate)
  A post-compile BIR pass deletes sync-free InstLdweights that reload
  the identical stationary operand, so each group pays ~1 weight load
  per K-chunk instead of one per matmul.
  Additive-attention score vectors are computed as fused DVE
  multiply-reduce against partition-broadcast vectors (loaded with a
  stride-0 broadcast DMA); u[i]+v[j] materializes as one rank-1 matmul
  (ones (x) v_row) with u riding the ScalarE Exp bias input. Gates are
  per-partition columns consumed directly by tensor_scalar ops.
  Sigmoid = 0.5*tanh(0.5x)+0.5 keeps ScalarE in one table set. The
  FiltrationGate projections fold on host: (txt@Wft)@wfg_t ==
  txt@(Wft@wfg_t), so Wft/Wfm are never shipped.
"""

import os
import numpy as np
import ml_dtypes

B, L, R, D = 8, 128, 49, 768
KC = D // 128  # 6
BF_NP = ml_dtypes.bfloat16

LAST = None  # BassKernelResults of the most recent run (for test harness)
LDW_DROPPED = 0
_CACHE = {}


def _pack_w(w):
    # (768, ncols) -> (128, KC*ncols): [p, kc*ncols + n] = w[kc*128 + p, n]
    ncols = w.shape[1]
    return np.ascontiguousarray(
        w.reshape(KC, 128, ncols).transpose(1, 0, 2).reshape(128, KC * ncols)
    ).astype(BF_NP)


def _dedup_ldweights(nc, mybir):
    """Drop sync-free InstLdweights that reload the PE stationary operand
    already resident from the previous load. Conservative: any intervening
    PE load (incl. transposes) resets the tracked state; lds carrying
    semaphore waits/updates are never dropped."""
    dropped = 0
    for blk in nc.m.functions[0].blocks:
        last_w = None
        new = []
        for i in blk.instructions:
            if getattr(i, "engine", None) == mybir.EngineType.PE and \
                    isinstance(i, mybir.InstLdweights):
                w = str(i.ins[0])
                si = i.sync_info
                clean = si is None or (not si.on_wait and not si.on_update)
                if w == last_w and clean:
                    dropped += 1
                    continue
                last_w = w
            new.append(i)
        blk.instructions = new
    return dropped


def _build(bias_flags):
    from contextlib import ExitStack
    import concourse.bass as bass
    import concourse.tile as tile
    from concourse import bacc, mybir
    from concourse.alu_op_type import AluOpType
    global LDW_DROPPED

    ga_bias, gb_bias, gc_bias, gd_bias, ge_bias = bias_flags

    F32 = mybir.dt.float32
    BF = mybir.dt.bfloat16
    AF = mybir.ActivationFunctionType
    MUL, ADD = AluOpType.mult, AluOpType.add

    nc = bacc.Bacc("TRN2", target_bir_lowering=False, debug=False,
                   enable_asserts=False)

    txt_d = nc.dram_tensor("txt", [L, D], BF, kind="ExternalInput").ap()
    vis_d = nc.dram_tensor("vis", [R, D], BF, kind="ExternalInput").ap()
    wA_d = nc.dram_tensor("wA", [128, KC * 2304], BF, kind="ExternalInput").ap()
    wV_d = nc.dram_tensor("wV", [128, KC * D], BF, kind="ExternalInput").ap()
    wB_d = nc.dram_tensor("wB", [128, KC * 1536], BF, kind="ExternalInput").ap()
    wC_d = nc.dram_tensor("wC", [128, KC * D], BF, kind="ExternalInput").ap()
    wD_d = nc.dram_tensor("wD", [128, KC * D], BF, kind="ExternalInput").ap()
    wE_d = nc.dram_tensor("wE", [128, KC * D], BF, kind="ExternalInput").ap()
    vbc_d = nc.dram_tensor("vbc", [128, 8 * D], BF, kind="ExternalInput").ap()
    id_d = nc.dram_tensor("identd", [128, 128], BF, kind="ExternalInput").ap()
    scal_d = nc.dram_tensor("scal", [128, 4], F32, kind="ExternalInput").ap()
    brows_d = nc.dram_tensor("brows", [1, 5376], BF, kind="ExternalInput").ap()
    out_d = nc.dram_tensor("out", [L, D], F32, kind="ExternalOutput").ap()

    # vbc blocks: 0 wa1_t, 1 wa1_i, 2 wa2_i, 3 wa2_t, 4 wg_i, 5 wg_t, 6 c_t, 7 c_m
    VB = lambda i: slice(i * D, (i + 1) * D)

    with tile.TileContext(nc) as tc, ExitStack() as ctx:
        const = ctx.enter_context(tc.tile_pool(name="const", bufs=1))
        wpool = ctx.enter_context(tc.tile_pool(name="wpool", bufs=1))
        acts = ctx.enter_context(tc.tile_pool(name="acts", bufs=1))
        tmp = ctx.enter_context(tc.tile_pool(name="tmp", bufs=2))
        psum = ctx.enter_context(tc.tile_pool(name="psum", bufs=1, space="PSUM"))
        psm = ctx.enter_context(tc.tile_pool(name="psm", bufs=3, space="PSUM"))

        # ---- DMAs. sync ring (HWDGE): txt + the big packs in consumption
        # order; gpsimd ring (SWDGE): vis/ident/wV/wC + broadcast + tiny.
        txt_bf = const.tile([L, D], BF, tag="txt")
        nc.sync.dma_start(out=txt_bf, in_=txt_d)
        wA_sb = wpool.tile([128, KC * 2304], BF, tag="wA")
        for c0 in range(0, KC, 2):
            nc.sync.dma_start(out=wA_sb[:, c0 * 2304:(c0 + 2) * 2304],
                              in_=wA_d[:, c0 * 2304:(c0 + 2) * 2304])
        vbc_sb = const.tile([128, 8 * D], BF, tag="vbc")
        nc.sync.dma_start(out=vbc_sb, in_=vbc_d)
        wV_sb = wpool.tile([128, KC * D], BF, tag="wV")
        nc.sync.dma_start(out=wV_sb, in_=wV_d)
        wB_sb = wpool.tile([128, KC * 1536], BF, tag="wB")
        h2 = KC // 2 * 1536
        nc.sync.dma_start(out=wB_sb[:, 0:h2], in_=wB_d[:, 0:h2])
        nc.sync.dma_start(out=wB_sb[:, h2:], in_=wB_d[:, h2:])
        wC_sb = wpool.tile([128, KC * D], BF, tag="wC")
        nc.sync.dma_start(out=wC_sb, in_=wC_d)
        wD_sb = wpool.tile([128, KC * D], BF, tag="wD")
        nc.sync.dma_start(out=wD_sb, in_=wD_d)
        wE_sb = wpool.tile([128, KC * D], BF, tag="wE")
        nc.sync.dma_start(out=wE_sb, in_=wE_d)

        vis_bf = const.tile([R, D], BF, tag="vis")
        nc.gpsimd.dma_start(out=vis_bf, in_=vis_d)
        ident = const.tile([128, 128], BF, tag="ident")
        nc.gpsimd.dma_start(out=ident, in_=id_d)
        scal_sb = const.tile([128, 4], F32, tag="scal")
        nc.gpsimd.dma_start(out=scal_sb, in_=scal_d)
        brows_sb = const.tile([1, 5376], BF, tag="brows")
        nc.gpsimd.dma_start(out=brows_sb, in_=brows_d)

        ones_row = const.tile([1, 128], BF, tag="ones")
        nc.vector.memset(ones_row, 1.0)

        # ---- transposes: txt^T, vis^T
        txtT = acts.tile([128, KC * 128], BF, tag="txtT")
        for kc in range(KC):
            ps = psm.tile([128, 128], BF, tag="sm")
            nc.tensor.transpose(ps, txt_bf[:, kc * 128:(kc + 1) * 128], ident)
            nc.vector.tensor_copy(txtT[:, kc * 128:(kc + 1) * 128], ps)
        visT = acts.tile([128, KC * R], BF, tag="visT")
        for kc in range(KC):
            ps = psm.tile([128, 128], BF, tag="sm")
            nc.tensor.transpose(ps[:, 0:R], vis_bf[:, kc * 128:(kc + 1) * 128],
                                ident[0:R, 0:R])
            nc.vector.tensor_copy(visT[:, kc * R:(kc + 1) * R], ps[:, 0:R])

        def group(ps_chunks, lhsT_src, lhsT_w, w_sb, wcols, bias_row):
            """One fused matmul group: for each kc, one stationary load of
            lhsT_src[:, kc chunk] reused across all moving chunks.
            ps_chunks: list of (psum_ap, col0, col1, opens, closes)."""
            for kc in range(KC):
                base = kc * wcols
                for ci, (pap, c0, c1, opens, closes) in enumerate(ps_chunks):
                    nc.tensor.matmul(
                        pap,
                        lhsT=lhsT_src[:, kc * lhsT_w:(kc + 1) * lhsT_w],
                        rhs=w_sb[:, base + c0: base + c1],
                        start=(opens and kc == 0),
                        stop=(closes and kc == KC - 1 and bias_row is None),
                    )
            if bias_row is not None:
                for ci, (pap, c0, c1, opens, closes) in enumerate(ps_chunks):
                    nc.tensor.matmul(
                        pap, lhsT=ones_row,
                        rhs=brows_sb[:, bias_row + c0: bias_row + c1],
                        start=False, stop=closes)

        def ttr(dst_col, in0, in1, parts=128):
            scr = tmp.tile([128, D], BF, tag="scr")
            nc.vector.scalar_tensor_tensor(
                out=scr[0:parts], in0=in0, scalar=1.0, in1=in1,
                op0=MUL, op1=MUL, accum_out=dst_col)

        def tanh_chunks(dst, ps, total, bias=0.0):
            for c0 in range(0, total, 512):
                c1 = min(c0 + 512, total)
                nc.scalar.activation(out=dst[:, c0:c1], in_=ps[:, c0:c1],
                                     func=AF.Tanh, bias=bias)

        # ---- GA: y1|y3|out_partial from txt^T
        out_ps = psum.tile([128, D], F32, tag="out")
        ya_ps = psum.tile([128, 1536], F32, tag="big")
        group([(ya_ps[:, 0:512], 0, 512, True, True),
               (ya_ps[:, 512:1024], 512, 1024, True, True),
               (ya_ps[:, 1024:1536], 1024, 1536, True, True),
               (out_ps[:, 0:512], 1536, 2048, True, False),
               (out_ps[:, 512:768], 2048, 2304, True, False)],
              txtT, 128, wA_sb, 2304, 0 if ga_bias else None)
        y13 = acts.tile([128, 1536], BF, tag="y13")
        tanh_chunks(y13, ya_ps, 1536)

        u1c = acts.tile([128, 1], F32, tag="u1c")
        ttr(u1c, y13[:, 0:D], vbc_sb[:, VB(0)])
        v2c = acts.tile([128, 1], F32, tag="v2c")
        ttr(v2c, y13[:, D:2 * D], vbc_sb[:, VB(3)])
        zf1 = acts.tile([128, 1], F32, tag="zf1")
        ttr(zf1, txt_bf, vbc_sb[:, VB(6)])

        # ---- GV: yv from vis^T
        gv_ps = psum.tile([128, 1536], F32, tag="big")
        group([(gv_ps[0:R, 0:512], 0, 512, True, True),
               (gv_ps[0:R, 512:768], 512, 768, True, True)],
              visT, R, wV_sb, D, None)
        yv = acts.tile([R, D], BF, tag="yv")
        for c0, c1 in ((0, 512), (512, 768)):
            nc.scalar.activation(out=yv[:, c0:c1], in_=gv_ps[0:R, c0:c1],
                                 func=AF.Tanh)
        v1c = acts.tile([R, 1], F32, tag="v1c")
        ttr(v1c, yv, vbc_sb[0:R, VB(1)], parts=R)

        # ---- scores1 = exp(v1[r] + u1[l] + ba1); softmax over r; probs1^T
        v1cb = acts.tile([R, 1], BF, tag="v1cb")
        nc.vector.tensor_copy(v1cb, v1c)
        ps_v1r = psm.tile([1, 128], BF, tag="sm")
        nc.tensor.transpose(ps_v1r[:, 0:R], v1cb, ident[0:R, 0:R])
        v1r = acts.tile([1, R], BF, tag="v1r")
        nc.vector.tensor_copy(v1r, ps_v1r[:, 0:R])
        u1b = acts.tile([128, 1], F32, tag="u1b")
        nc.vector.tensor_scalar_add(u1b, u1c, scal_sb[:, 0:1])

        s1_ps = psm.tile([128, R], F32, tag="sm")
        nc.tensor.matmul(s1_ps, lhsT=ones_row, rhs=v1r, start=True, stop=True)
        probs1 = acts.tile([128, R], F32, tag="p1")
        nc.scalar.activation(out=probs1, in_=s1_ps, func=AF.Exp, bias=u1b)
        rs1 = acts.tile([128, 1], F32, tag="rs1")
        nc.vector.reduce_sum(rs1, probs1, axis=mybir.AxisListType.X)
        rr1 = acts.tile([128, 1], F32, tag="rr1")
        nc.vector.reciprocal(rr1, rs1)
        p1b = acts.tile([128, R], BF, tag="p1b")
        nc.vector.tensor_scalar_mul(p1b, probs1, rr1)
        ps_p1t = psm.tile([R, 128], BF, tag="sm")
        nc.tensor.transpose(ps_p1t, p1b, ident)
        p1T = acts.tile([R, 128], BF, tag="p1T")
        nc.vector.tensor_copy(p1T, ps_p1t)

        # ---- att_img^T via PV (output lands transposed)
        aimgT = acts.tile([128, KC * 128], BF, tag="aimgT")
        for mc in range(KC):
            ps = psm.tile([128, 128], F32, tag="sm")
            nc.tensor.matmul(ps, lhsT=vis_bf[:, mc * 128:(mc + 1) * 128],
                             rhs=p1T, start=True, stop=True)
            nc.vector.tensor_copy(aimgT[:, mc * 128:(mc + 1) * 128], ps)

        # ---- GB: y2|new_img from att_img^T
        yb_ps = psum.tile([128, 1536], F32, tag="big")
        group([(yb_ps[:, 0:512], 0, 512, True, True),
               (yb_ps[:, 512:1024], 512, 1024, True, True),
               (yb_ps[:, 1024:1536], 1024, 1536, True, True)],
              aimgT, 128, wB_sb, 1536, 2304 if gb_bias else None)
        y2ni = acts.tile([128, 1536], BF, tag="y2ni")
        tanh_chunks(y2ni, yb_ps, 1536)
        ni = y2ni[:, D:2 * D]

        u2c = acts.tile([128, 1], F32, tag="u2c")
        ttr(u2c, y2ni[:, 0:D], vbc_sb[:, VB(2)])
        u2b = acts.tile([128, 1], F32, tag="u2b")
        nc.vector.tensor_scalar_add(u2b, u2c, scal_sb[:, 1:2])
        zgi = acts.tile([128, 1], F32, tag="zgi")
        ttr(zgi, ni, vbc_sb[:, VB(4)])

        # ---- scores2 = exp(v2[j] + u2[i] + ba2); softmax over j; probs2^T
        v2cb = acts.tile([128, 1], BF, tag="v2cb")
        nc.vector.tensor_copy(v2cb, v2c)
        ps_v2r = psm.tile([1, 128], BF, tag="sm")
        nc.tensor.transpose(ps_v2r, v2cb, ident)
        v2r = acts.tile([1, 128], BF, tag="v2r")
        nc.vector.tensor_copy(v2r, ps_v2r)

        s2_ps = psm.tile([128, 128], F32, tag="sm")
        nc.tensor.matmul(s2_ps, lhsT=ones_row, rhs=v2r, start=True, stop=True)
        probs2 = acts.tile([128, 128], F32, tag="p2")
        nc.scalar.activation(out=probs2, in_=s2_ps, func=AF.Exp, bias=u2b)
        rs2 = acts.tile([128, 1], F32, tag="rs2")
        nc.vector.reduce_sum(rs2, probs2, axis=mybir.AxisListType.X)
        rr2 = acts.tile([128, 1], F32, tag="rr2")
        nc.vector.reciprocal(rr2, rs2)
        p2b = acts.tile([128, 128], BF, tag="p2b")
        nc.vector.tensor_scalar_mul(p2b, probs2, rr2)
        ps_p2t = psm.tile([128, 128], BF, tag="sm")
        nc.tensor.transpose(ps_p2t, p2b, ident)
        p2T = acts.tile([128, 128], BF, tag="p2T")
        nc.vector.tensor_copy(p2T, ps_p2t)

        # ---- att_text^T via PV2
        atxtT = acts.tile([128, KC * 128], BF, tag="atxtT")
        for mc in range(KC):
            ps = psm.tile([128, 128], F32, tag="sm")
            nc.tensor.matmul(ps, lhsT=txt_bf[:, mc * 128:(mc + 1) * 128],
                             rhs=p2T, start=True, stop=True)
            nc.vector.tensor_copy(atxtT[:, mc * 128:(mc + 1) * 128], ps)

        # ---- GC: new_txt from att_text^T
        gc_ps = psum.tile([128, 1536], F32, tag="big")
        group([(gc_ps[:, 0:512], 0, 512, True, True),
               (gc_ps[:, 512:768], 512, 768, True, True)],
              atxtT, 128, wC_sb, D, 3840 if gc_bias else None)
        nt = acts.tile([128, D], BF, tag="nt")
        tanh_chunks(nt, gc_ps, D)

        # ---- GMF gate (per-partition column) + multimodal
        zgt = acts.tile([128, 1], F32, tag="zgt")
        ttr(zgt, nt, vbc_sb[:, VB(5)])
        zg = acts.tile([128, 1], F32, tag="zg")
        nc.vector.tensor_add(zg, zgi, zgt)
        tg = acts.tile([128, 1], F32, tag="tg")
        nc.scalar.activation(out=tg, in_=zg, func=AF.Tanh, scale=0.5)
        g_col = acts.tile([128, 1], F32, tag="gcol")
        nc.vector.tensor_scalar(g_col, tg, 0.5, 0.5, MUL, ADD)

        # multimodal assembled per 128-chunk so mm^T transposes pipeline
        mm_nat = acts.tile([128, D], BF, tag="mmn")
        mmT = acts.tile([128, KC * 128], BF, tag="mmT")
        for kc in range(KC):
            sl = slice(kc * 128, (kc + 1) * 128)
            dmm = tmp.tile([128, 128], BF, tag="dmm")
            nc.vector.tensor_sub(dmm, ni[:, sl], nt[:, sl])
            dms = tmp.tile([128, 128], BF, tag="dms")
            nc.vector.tensor_scalar_mul(dms, dmm, g_col)
            nc.vector.tensor_add(mm_nat[:, sl], nt[:, sl], dms)
            ps = psm.tile([128, 128], BF, tag="sm")
            nc.tensor.transpose(ps, mm_nat[:, sl], ident)
            nc.vector.tensor_copy(mmT[:, sl], ps)

        # ---- FiltrationGate column (host-folded c_t, c_m)
        zf2 = acts.tile([128, 1], F32, tag="zf2")
        ttr(zf2, mm_nat, vbc_sb[:, VB(7)])
        zf = acts.tile([128, 1], F32, tag="zf")
        nc.vector.tensor_add(zf, zf1, zf2)
        tf = acts.tile([128, 1], F32, tag="tf")
        nc.scalar.activation(out=tf, in_=zf, func=AF.Tanh, scale=0.5,
                             bias=scal_sb[:, 2:3])
        f_col = acts.tile([128, 1], F32, tag="fcol")
        nc.vector.tensor_scalar(f_col, tf, 0.5, 0.5, MUL, ADD)

        # ---- GD: rv from mm^T ; reserved = f * rv
        gd_ps = psum.tile([128, 1536], F32, tag="big")
        group([(gd_ps[:, 0:512], 0, 512, True, True),
               (gd_ps[:, 512:768], 512, 768, True, True)],
              mmT, 128, wD_sb, D, 4608 if gd_bias else None)
        rv = acts.tile([128, D], BF, tag="rv")
        tanh_chunks(rv, gd_ps, D)
        res = acts.tile([128, D], BF, tag="res")
        resT = acts.tile([128, KC * 128], BF, tag="resT")
        for kc in range(KC):
            sl = slice(kc * 128, (kc + 1) * 128)
            nc.vector.tensor_scalar_mul(res[:, sl], rv[:, sl], f_col)
            ps = psm.tile([128, 128], BF, tag="sm")
            nc.tensor.transpose(ps, res[:, sl], ident)
            nc.vector.tensor_copy(resT[:, sl], ps)

        # ---- GE: accumulate res@Wout_m into out_ps (+ bout), write out
        group([(out_ps[:, 0:512], 0, 512, False, True),
               (out_ps[:, 512:768], 512, 768, False, True)],
              resT, 128, wE_sb, D, None if not ge_bias else 1536)
        out_sb = acts.tile([L, D], F32, tag="outsb")
        for c0, c1 in ((0, 512), (512, 768)):
            nc.vector.tensor_copy(out_sb[:, c0:c1], out_ps[:, c0:c1])
            nc.sync.dma_start(out=out_d[:, c0:c1], in_=out_sb[:, c0:c1])

    nc.compile()
    LDW_DROPPED = _dedup_ldweights(nc, mybir)
    return nc


def _inputs_pack(inp):
    f32 = np.float32
    g = lambda k: np.asarray(inp[k], dtype=f32)

    wA = _pack_w(np.concatenate([g("Wt1"), g("Wt2"), g("Wout_t")], axis=1))
    wV = _pack_w(g("Wi1"))
    wB = _pack_w(np.concatenate([g("Wi2"), g("Wgi")], axis=1))
    wC = _pack_w(g("Wgt"))
    wD = _pack_w(g("Wrv"))
    wE = _pack_w(g("Wout_m"))

    c_t = g("Wft").astype(np.float64) @ g("wfg_t").astype(np.float64)
    c_m = g("Wfm").astype(np.float64) @ g("wfg_m").astype(np.float64)
    s_fh = 0.5 * (float(g("bfm").astype(np.float64) @ g("wfg_m").astype(np.float64))
                  + float(g("bfg")))

    vbc = np.concatenate([g("wa1_t"), g("wa1_i"), g("wa2_i"), g("wa2_t"),
                          g("wg_i"), g("wg_t"),
                          c_t.astype(f32), c_m.astype(f32)]).reshape(1, 8 * D)
    vbc = np.ascontiguousarray(np.repeat(vbc, 128, axis=0)).astype(BF_NP)

    scal = np.zeros((128, 4), f32)
    scal[:, 0] = float(g("ba1"))
    scal[:, 1] = float(g("ba2"))
    scal[:, 2] = s_fh

    brows = np.zeros((1, 5376), f32)
    brows[0, 0:768] = g("bt1")
    brows[0, 1536:2304] = g("bout")
    brows[0, 2304:3072] = g("bi2")
    brows[0, 3072:3840] = g("bgi")
    brows[0, 3840:4608] = g("bgt")
    brows[0, 4608:5376] = g("brv")
    bias_flags = (
        bool(np.any(g("bt1")) or np.any(g("bout"))),  # ga (bt1; bout w/ GE)
        bool(np.any(g("bi2")) or np.any(g("bgi"))),   # gb
        bool(np.any(g("bgt"))),                        # gc
        bool(np.any(g("brv"))),                        # gd
        False,                                         # ge (bout rides GA)
    )
    # bout rides GA's bias row range [1536:2304] only if ga_bias; if only
    # bout is nonzero, GE emits it from brows[1536:2304] via ge flag.
    brows = brows.astype(BF_NP)

    ident = np.eye(128, dtype=BF_NP)

    shared = dict(wA=wA, wV=wV, wB=wB, wC=wC, wD=wD, wE=wE, vbc=vbc,
                  identd=ident, scal=scal, brows=brows)

    txt = g("txt_hidden").astype(BF_NP)
    vis = g("vis_hidden").astype(BF_NP)
    in_maps = []
    for c in range(B):
        m = dict(shared)
        m["txt"] = np.ascontiguousarray(txt[c])
        m["vis"] = np.ascontiguousarray(vis[c])
        in_maps.append(m)
    return in_maps, bias_flags


def kernel(**inputs):
    global LAST
    from concourse import bass_utils

    in_maps, bias_flags = _inputs_pack(inputs)
    key = ("v2", bias_flags)
    nc = _CACHE.get(key)
    if nc is None:
        nc = _build(bias_flags)
        _CACHE[key] = nc

    res = bass_utils.run_bass_kernel_spmd(
        nc, in_maps, core_ids=list(range(B)),
        trace=bool(os.environ.get("KERNEL_TRACE")),
    )
    LAST = res
    out = np.stack([np.asarray(res.results[c]["out"]) for c in range(B)], axis=0)
    return out.astype(np.float32)


# revision 14
# speedup vs baseline: 1.6146x; 1.1652x over previous
"""Trainium2 Bass kernel for nn_AdaptiveCoFusion (B=8, L=128, R=49, D=768).

Pure data parallel: one batch element per NeuronCore (8 cores), weights
replicated, host-packed to bf16 in SBUF layout.

Key mathematical identity: the reference's additive (Bahdanau) attention
scores are separable, scores[q, k] = u[q] + v[k], so the softmax over k
is INDEPENDENT of the query term u: softmax_k(u[q] + v[k]) = softmax(v).
Both attention matrices are therefore constant across queries:
    att_img[l, :]  = softmax(v1) @ vis   (one D-vector)
    att_text[i, :] = softmax(v2) @ txt   (one D-vector)
which collapses the GMF gate to a scalar, multimodal to a D-vector,
reserved to the outer product fgate (x) tanh(mm@Wrv + brv), and
    output = txt @ Wout_t + fgate (x) (rv @ Wout_m) + bout.
Wt1, Wi2, wa1_t, wa2_i, bt1, bi2, ba1, ba2 drop out exactly. The kernel
computes, per core: txt@[Wt2|Wout_t] (fused stationary-txt^T group),
vis@Wi1, two softmaxes over score vectors, four vector-matrix products
(moving weights, M=1 stationary vector columns), a handful of tiny PE
dot products / broadcasts, and one rank-1 update of the output PSUM.
Sigmoids are 0.5*tanh(0.5x)+0.5 (single ScalarE table set);
(txt@Wft)@wfg_t folds on host to txt@(Wft@wfg_t). A post-compile BIR
pass drops sync-free InstLdweights that reload the identical stationary
operand (the fused txt^T group loads each chunk once for 4 matmuls).
"""

import os
import numpy as np
import ml_dtypes

B, L, R, D = 8, 128, 49, 768
KC = D // 128  # 6
BF_NP = ml_dtypes.bfloat16

LAST = None  # BassKernelResults of the most recent run (for test harness)
LDW_DROPPED = 0
_CACHE = {}


def _pack_w(w):
    # (768, ncols) -> (128, KC*ncols): [p, kc*ncols + n] = w[kc*128 + p, n]
    ncols = w.shape[1]
    return np.ascontiguousarray(
        w.reshape(KC, 128, ncols).transpose(1, 0, 2).reshape(128, KC * ncols)
    ).astype(BF_NP)


def _pack_col(v):
    # (768,) -> (128, KC): [p, kc] = v[kc*128 + p]
    return np.ascontiguousarray(v.reshape(KC, 128).T)


def _dedup_ldweights(nc, mybir):
    """Drop sync-free InstLdweights that reload the PE stationary operand
    already resident from the previous load."""
    dropped = 0
    for blk in nc.m.functions[0].blocks:
        last_w = None
        new = []
        for i in blk.instructions:
            if getattr(i, "engine", None) == mybir.EngineType.PE and \
                    isinstance(i, mybir.InstLdweights):
                w = str(i.ins[0])
                si = i.sync_info
                clean = si is None or (not si.on_wait and not si.on_update)
                if w == last_w and clean:
                    dropped += 1
                    continue
                last_w = w
            new.append(i)
        blk.instructions = new
    return dropped


def _build(bias_flags):
    from contextlib import ExitStack
    import concourse.bass as bass  # noqa: F401
    import concourse.tile as tile
    from concourse import bacc, mybir
    from concourse.alu_op_type import AluOpType
    global LDW_DROPPED

    gt_bias, gi_bias, rv_bias, out_bias = bias_flags

    F32 = mybir.dt.float32
    BF = mybir.dt.bfloat16
    AF = mybir.ActivationFunctionType
    MUL, ADD = AluOpType.mult, AluOpType.add

    nc = bacc.Bacc("TRN2", target_bir_lowering=False, debug=False,
                   enable_asserts=False)

    txt_d = nc.dram_tensor("txt", [L, D], BF, kind="ExternalInput").ap()
    vis_d = nc.dram_tensor("vis", [R, D], BF, kind="ExternalInput").ap()
    wP_d = nc.dram_tensor("wP", [128, KC * 1536], BF, kind="ExternalInput").ap()
    wI1_d = nc.dram_tensor("wI1", [128, KC * D], BF, kind="ExternalInput").ap()
    wGT_d = nc.dram_tensor("wGT", [128, KC * D], BF, kind="ExternalInput").ap()
    wGI_d = nc.dram_tensor("wGI", [128, KC * D], BF, kind="ExternalInput").ap()
    wRV_d = nc.dram_tensor("wRV", [128, KC * D], BF, kind="ExternalInput").ap()
    wOM_d = nc.dram_tensor("wOM", [128, KC * D], BF, kind="ExternalInput").ap()
    vbc_d = nc.dram_tensor("vbc", [128, 3 * D], BF, kind="ExternalInput").ap()
    cols_d = nc.dram_tensor("colsd", [128, 18], BF, kind="ExternalInput").ap()
    id_d = nc.dram_tensor("identd", [128, 128], BF, kind="ExternalInput").ap()
    scal_d = nc.dram_tensor("scal", [1, 4], F32, kind="ExternalInput").ap()
    brow_d = nc.dram_tensor("brow", [1, 4 * D], BF, kind="ExternalInput").ap()
    out_d = nc.dram_tensor("out", [L, D], F32, kind="ExternalOutput").ap()

    # vbc blocks (128-bcast): 0=wa2_t, 1=c_t, 2=wa1_i (rows 0:R used)
    # cols: [0:6]=wg_i, [6:12]=wg_t, [12:18]=c_m   (column form)
    # brow rows: [0:768]=bgt, [768:1536]=bgi, [1536:2304]=brv, [2304:3072]=bout
    # scal: [0]=0.5*bg, [1]=s_f (bfm@wfg_m+bfg)
    VB = lambda i: slice(i * D, (i + 1) * D)

    with tile.TileContext(nc) as tc, ExitStack() as ctx:
        const = ctx.enter_context(tc.tile_pool(name="const", bufs=1))
        wpool = ctx.enter_context(tc.tile_pool(name="wpool", bufs=1))
        acts = ctx.enter_context(tc.tile_pool(name="acts", bufs=1))
        tmp = ctx.enter_context(tc.tile_pool(name="tmp", bufs=2))
        pso = ctx.enter_context(tc.tile_pool(name="pso", bufs=1, space="PSUM"))
        psb = ctx.enter_context(tc.tile_pool(name="psb", bufs=1, space="PSUM"))
        psr = ctx.enter_context(tc.tile_pool(name="psr", bufs=1, space="PSUM"))
        psm = ctx.enter_context(tc.tile_pool(name="psm", bufs=2, space="PSUM"))

        # ---- DMAs: sync ring = big streams in consumption order
        txt_bf = const.tile([L, D], BF, tag="txt")
        nc.sync.dma_start(out=txt_bf, in_=txt_d)
        wP_sb = wpool.tile([128, KC * 1536], BF, tag="wP")
        hp = 3 * 1536
        nc.sync.dma_start(out=wP_sb[:, 0:hp], in_=wP_d[:, 0:hp])
        nc.sync.dma_start(out=wP_sb[:, hp:], in_=wP_d[:, hp:])
        wI1_sb = wpool.tile([128, KC * D], BF, tag="wI1")
        nc.sync.dma_start(out=wI1_sb, in_=wI1_d)
        wGT_sb = wpool.tile([128, KC * D], BF, tag="wGT")
        nc.sync.dma_start(out=wGT_sb, in_=wGT_d)
        wGI_sb = wpool.tile([128, KC * D], BF, tag="wGI")
        nc.sync.dma_start(out=wGI_sb, in_=wGI_d)
        wRV_sb = wpool.tile([128, KC * D], BF, tag="wRV")
        nc.sync.dma_start(out=wRV_sb, in_=wRV_d)
        wOM_sb = wpool.tile([128, KC * D], BF, tag="wOM")
        nc.sync.dma_start(out=wOM_sb, in_=wOM_d)

        # gpsimd ring (SWDGE): small tensors
        vis_bf = const.tile([R, D], BF, tag="vis")
        nc.gpsimd.dma_start(out=vis_bf, in_=vis_d)
        ident = const.tile([128, 128], BF, tag="ident")
        nc.gpsimd.dma_start(out=ident, in_=id_d)
        vbc_sb = const.tile([128, 3 * D], BF, tag="vbc")
        nc.gpsimd.dma_start(out=vbc_sb, in_=vbc_d)
        cols_sb = const.tile([128, 18], BF, tag="cols")
        nc.gpsimd.dma_start(out=cols_sb, in_=cols_d)
        scal_sb = const.tile([1, 4], F32, tag="scal")
        nc.gpsimd.dma_start(out=scal_sb, in_=scal_d)
        brow_sb = const.tile([1, 4 * D], BF, tag="brow")
        nc.gpsimd.dma_start(out=brow_sb, in_=brow_d)

        ones_row = const.tile([1, 128], BF, tag="ones")
        nc.vector.memset(ones_row, 1.0)
        ones_c128 = const.tile([128, 1], BF, tag="onesc")
        nc.vector.memset(ones_c128, 1.0)
        one11 = ones_row[:, 0:1]

        def fused_reduce(dst_col, in0, in1, parts=128):
            scr = tmp.tile([128, D], BF, tag="scr")
            nc.vector.scalar_tensor_tensor(
                out=scr[0:parts], in0=in0, scalar=1.0, in1=in1,
                op0=MUL, op1=MUL, accum_out=dst_col)

        # ---- transposes
        txtT = acts.tile([128, KC * 128], BF, tag="txtT")
        for kc in range(KC):
            ps = psm.tile([128, 128], BF, tag="sm")
            nc.tensor.transpose(ps, txt_bf[:, kc * 128:(kc + 1) * 128], ident)
            nc.vector.tensor_copy(txtT[:, kc * 128:(kc + 1) * 128], ps)
        visT = acts.tile([128, KC * R], BF, tag="visT")
        for kc in range(KC):
            ps = psm.tile([128, 128], BF, tag="sm")
            nc.tensor.transpose(ps[:, 0:R], vis_bf[:, kc * 128:(kc + 1) * 128],
                                ident[0:R, 0:R])
            nc.vector.tensor_copy(visT[:, kc * R:(kc + 1) * R], ps[:, 0:R])

        # ---- big fused group: yt = txt@Wt2 ; out_ps += txt@Wout_t
        out_ps = pso.tile([128, D], F32, tag="out")
        yt_ps = psb.tile([128, D], F32, tag="big")
        for kc in range(KC):
            base = kc * 1536
            lhsT = txtT[:, kc * 128:(kc + 1) * 128]
            nc.tensor.matmul(yt_ps[:, 0:512], lhsT=lhsT,
                             rhs=wP_sb[:, base:base + 512],
                             start=(kc == 0), stop=(kc == KC - 1))
            nc.tensor.matmul(yt_ps[:, 512:768], lhsT=lhsT,
                             rhs=wP_sb[:, base + 512:base + 768],
                             start=(kc == 0), stop=(kc == KC - 1))
            nc.tensor.matmul(out_ps[:, 0:512], lhsT=lhsT,
                             rhs=wP_sb[:, base + 768:base + 1280],
                             start=(kc == 0), stop=False)
            nc.tensor.matmul(out_ps[:, 512:768], lhsT=lhsT,
                             rhs=wP_sb[:, base + 1280:base + 1536],
                             start=(kc == 0), stop=False)
        y3 = acts.tile([128, D], BF, tag="y3")
        for c0, c1 in ((0, 512), (512, 768)):
            nc.scalar.activation(out=y3[:, c0:c1], in_=yt_ps[:, c0:c1],
                                 func=AF.Tanh)
        v2c = acts.tile([128, 1], F32, tag="v2c")
        fused_reduce(v2c, y3, vbc_sb[:, VB(0)])
        zf1 = acts.tile([128, 1], F32, tag="zf1")
        fused_reduce(zf1, txt_bf, vbc_sb[:, VB(1)])

        # ---- vis branch: yv = tanh(vis@Wi1) ; v1
        gv_ps = psb.tile([128, D], F32, tag="big")
        for kc in range(KC):
            lhsT = visT[:, kc * R:(kc + 1) * R]
            nc.tensor.matmul(gv_ps[0:R, 0:512], lhsT=lhsT,
                             rhs=wI1_sb[:, kc * D:kc * D + 512],
                             start=(kc == 0), stop=(kc == KC - 1))
            nc.tensor.matmul(gv_ps[0:R, 512:768], lhsT=lhsT,
                             rhs=wI1_sb[:, kc * D + 512:kc * D + 768],
                             start=(kc == 0), stop=(kc == KC - 1))
        yv = acts.tile([R, D], BF, tag="yv")
        for c0, c1 in ((0, 512), (512, 768)):
            nc.scalar.activation(out=yv[:, c0:c1], in_=gv_ps[0:R, c0:c1],
                                 func=AF.Tanh)
        v1c = acts.tile([R, 1], F32, tag="v1c")
        fused_reduce(v1c, yv, vbc_sb[0:R, VB(2)], parts=R)

        def softmax_col(vcol, parts):
            """exp / partition-sum / scale for a (parts,1) score column.
            Returns normalized bf16 (parts,1) probabilities."""
            e = acts.tile([parts, 1], F32, tag=f"e{parts}")
            nc.scalar.activation(out=e, in_=vcol, func=AF.Exp)
            eb = acts.tile([parts, 1], BF, tag=f"eb{parts}")
            nc.vector.tensor_copy(eb, e)
            s_ps = psm.tile([1, 1], F32, tag="sm")
            nc.tensor.matmul(s_ps, lhsT=eb, rhs=ones_c128[0:parts],
                             start=True, stop=True)
            r = acts.tile([1, 1], F32, tag=f"r{parts}")
            nc.vector.reciprocal(r, s_ps)
            rb = acts.tile([1, 1], BF, tag=f"rb{parts}")
            nc.vector.tensor_copy(rb, r)
            rb_ps = psm.tile([128, 1], F32, tag="sm")
            nc.tensor.matmul(rb_ps[0:parts], lhsT=ones_row[:, 0:parts],
                             rhs=rb, start=True, stop=True)
            rbc = acts.tile([parts, 1], BF, tag=f"rbc{parts}")
            nc.vector.tensor_copy(rbc, rb_ps[0:parts])
            p = acts.tile([parts, 1], BF, tag=f"p{parts}")
            nc.vector.tensor_mul(p, eb, rbc)
            return p

        p1 = softmax_col(v1c, R)
        p2 = softmax_col(v2c, 128)

        # ---- attended vectors as (128, KC) columns: a[mc] = srcT-chunk @ p
        aimg_col = acts.tile([128, KC], BF, tag="aimg")
        for mc in range(KC):
            ps = psm.tile([128, 1], F32, tag="sm")
            nc.tensor.matmul(ps, lhsT=vis_bf[:, mc * 128:(mc + 1) * 128],
                             rhs=p1, start=True, stop=True)
            nc.vector.tensor_copy(aimg_col[:, mc:mc + 1], ps)
        atxt_col = acts.tile([128, KC], BF, tag="atxt")
        for mc in range(KC):
            ps = psm.tile([128, 1], F32, tag="sm")
            nc.tensor.matmul(ps, lhsT=txt_bf[:, mc * 128:(mc + 1) * 128],
                             rhs=p2, start=True, stop=True)
            nc.vector.tensor_copy(atxt_col[:, mc:mc + 1], ps)

        def vecmat_row(col_src, w_sb, bias_off, func, row_tag):
            """(1,D) row = func(vec @ W + b): vec as (128,KC) columns is the
            M=1 stationary; W pack chunks are the moving operand."""
            ps = psr.tile([1, D], F32, tag="row")
            for kc in range(KC):
                lhsT = col_src[:, kc:kc + 1]
                nc.tensor.matmul(ps[:, 0:512], lhsT=lhsT,
                                 rhs=w_sb[:, kc * D:kc * D + 512],
                                 start=(kc == 0),
                                 stop=(kc == KC - 1 and bias_off is None))
                nc.tensor.matmul(ps[:, 512:768], lhsT=lhsT,
                                 rhs=w_sb[:, kc * D + 512:kc * D + 768],
                                 start=(kc == 0),
                                 stop=(kc == KC - 1 and bias_off is None))
            if bias_off is not None:
                nc.tensor.matmul(ps[:, 0:512], lhsT=one11,
                                 rhs=brow_sb[:, bias_off:bias_off + 512],
                                 start=False, stop=True)
                nc.tensor.matmul(ps[:, 512:768], lhsT=one11,
                                 rhs=brow_sb[:, bias_off + 512:bias_off + 768],
                                 start=False, stop=True)
            row = acts.tile([1, D], BF, tag=row_tag)
            if func is None:
                nc.vector.tensor_copy(row, ps)
            else:
                nc.scalar.activation(out=row, in_=ps, func=func)
            return row

        def row_to_cols(row, col_tag):
            col = acts.tile([128, KC], BF, tag=col_tag)
            for mc in range(KC):
                ps = psm.tile([128, 1], BF, tag="sm")
                nc.tensor.transpose(ps, row[:, mc * 128:(mc + 1) * 128],
                                    ident[0:1, 0:1])
                nc.vector.tensor_copy(col[:, mc:mc + 1], ps)
            return col

        # ---- GMF vector stages
        nt_row = vecmat_row(atxt_col, wGT_sb, 0 if gt_bias else None,
                            AF.Tanh, "ntr")
        nt_col = row_to_cols(nt_row, "ntc")
        ni_row = vecmat_row(aimg_col, wGI_sb, 768 if gi_bias else None,
                            AF.Tanh, "nir")
        ni_col = row_to_cols(ni_row, "nic")

        # gate scalar: sigma(ni.wg_i + nt.wg_t + bg) via PE dots
        g_ps = psm.tile([1, 1], F32, tag="sm")
        for kc in range(KC):
            nc.tensor.matmul(g_ps, lhsT=ni_col[:, kc:kc + 1],
                             rhs=cols_sb[:, kc:kc + 1],
                             start=(kc == 0), stop=False)
        for kc in range(KC):
            nc.tensor.matmul(g_ps, lhsT=nt_col[:, kc:kc + 1],
                             rhs=cols_sb[:, 6 + kc:7 + kc],
                             start=False, stop=(kc == KC - 1))
        tg = acts.tile([1, 1], F32, tag="tg")
        nc.scalar.activation(out=tg, in_=g_ps, func=AF.Tanh, scale=0.5,
                             bias=scal_sb[:, 0:1])
        g11 = acts.tile([1, 1], BF, tag="g11")
        nc.vector.tensor_scalar(g11, tg, 0.5, 0.5, MUL, ADD)
        gb_ps = psm.tile([128, 1], F32, tag="sm")
        nc.tensor.matmul(gb_ps, lhsT=ones_row, rhs=g11, start=True, stop=True)
        g_col = acts.tile([128, 1], F32, tag="gcol")
        nc.vector.tensor_copy(g_col, gb_ps)

        # multimodal vector (columns)
        mmv_col = acts.tile([128, KC], BF, tag="mmv")
        dmm = tmp.tile([128, KC], BF, tag="dmm")
        nc.vector.tensor_sub(dmm, ni_col, nt_col)
        dms = tmp.tile([128, KC], BF, tag="dms")
        nc.vector.tensor_scalar_mul(dms, dmm, g_col)
        nc.vector.tensor_add(mmv_col, nt_col, dms)

        # ---- FiltrationGate column: sigma(txt@c_t + mmv.c_m + s_f)
        cm_ps = psm.tile([1, 1], F32, tag="sm")
        for kc in range(KC):
            nc.tensor.matmul(cm_ps, lhsT=mmv_col[:, kc:kc + 1],
                             rhs=cols_sb[:, 12 + kc:13 + kc],
                             start=(kc == 0), stop=(kc == KC - 1))
        hd = acts.tile([1, 1], F32, tag="hd")
        nc.vector.tensor_scalar(hd, cm_ps, scal_sb[:, 1:2], 0.5, ADD, MUL)
        hdb = acts.tile([1, 1], BF, tag="hdb")
        nc.vector.tensor_copy(hdb, hd)
        hb_ps = psm.tile([128, 1], F32, tag="sm")
        nc.tensor.matmul(hb_ps, lhsT=ones_row, rhs=hdb, start=True, stop=True)
        h_col = acts.tile([128, 1], F32, tag="hcol")
        nc.vector.tensor_copy(h_col, hb_ps)
        tf = acts.tile([128, 1], F32, tag="tf")
        nc.scalar.activation(out=tf, in_=zf1, func=AF.Tanh, scale=0.5,
                             bias=h_col)
        f_col = acts.tile([128, 1], BF, tag="fcol")
        nc.vector.tensor_scalar(f_col, tf, 0.5, 0.5, MUL, ADD)
        fr_ps = psm.tile([1, 128], BF, tag="sm")
        nc.tensor.transpose(fr_ps, f_col, ident)
        f_row = acts.tile([1, 128], BF, tag="frow")
        nc.vector.tensor_copy(f_row, fr_ps)

        # ---- reserved vector: rv = tanh(mmv@Wrv + brv); wov = rv@Wout_m
        rv_row = vecmat_row(mmv_col, wRV_sb, 1536 if rv_bias else None,
                            AF.Tanh, "rvr")
        rv_col = row_to_cols(rv_row, "rvc")
        wov_row = vecmat_row(rv_col, wOM_sb, None, None, "wov")

        # ---- out += f_col (x) wov_row (+ bout); copy; DMA
        nc.tensor.matmul(out_ps[:, 0:512], lhsT=f_row,
                         rhs=wov_row[:, 0:512], start=False,
                         stop=(not out_bias))
        nc.tensor.matmul(out_ps[:, 512:768], lhsT=f_row,
                         rhs=wov_row[:, 512:768], start=False,
                         stop=(not out_bias))
        if out_bias:
            nc.tensor.matmul(out_ps[:, 0:512], lhsT=one11,
                             rhs=brow_sb[:, 2304:2816], start=False, stop=True)
            nc.tensor.matmul(out_ps[:, 512:768], lhsT=one11,
                             rhs=brow_sb[:, 2816:3072], start=False, stop=True)
        out_sb = acts.tile([L, D], F32, tag="outsb")
        for c0, c1 in ((0, 512), (512, 768)):
            nc.vector.tensor_copy(out_sb[:, c0:c1], out_ps[:, c0:c1])
            nc.sync.dma_start(out=out_d[:, c0:c1], in_=out_sb[:, c0:c1])

    nc.compile()
    LDW_DROPPED = _dedup_ldweights(nc, mybir)
    return nc


def _inputs_pack(inp):
    f32 = np.float32
    g = lambda k: np.asarray(inp[k], dtype=f32)

    wP = _pack_w(np.concatenate([g("Wt2"), g("Wout_t")], axis=1))
    wI1 = _pack_w(g("Wi1"))
    wGT = _pack_w(g("Wgt"))
    wGI = _pack_w(g("Wgi"))
    wRV = _pack_w(g("Wrv"))
    wOM = _pack_w(g("Wout_m"))

    c_t = g("Wft").astype(np.float64) @ g("wfg_t").astype(np.float64)
    c_m = g("Wfm").astype(np.float64) @ g("wfg_m").astype(np.float64)
    s_f = float(g("bfm").astype(np.float64) @ g("wfg_m").astype(np.float64)) \
        + float(g("bfg"))

    vbc = np.concatenate([g("wa2_t"), c_t.astype(f32),
                          g("wa1_i")]).reshape(1, 3 * D)
    vbc = np.ascontiguousarray(np.repeat(vbc, 128, axis=0)).astype(BF_NP)

    cols = np.zeros((128, 18), f32)
    cols[:, 0:6] = _pack_col(g("wg_i"))
    cols[:, 6:12] = _pack_col(g("wg_t"))
    cols[:, 12:18] = _pack_col(c_m.astype(f32))
    cols = cols.astype(BF_NP)

    scal = np.zeros((1, 4), f32)
    scal[0, 0] = 0.5 * float(g("bg"))
    scal[0, 1] = s_f

    brow = np.zeros((1, 4 * D), f32)
    brow[0, 0:768] = g("bgt")
    brow[0, 768:1536] = g("bgi")
    brow[0, 1536:2304] = g("brv")
    brow[0, 2304:3072] = g("bout")
    bias_flags = (bool(np.any(g("bgt"))), bool(np.any(g("bgi"))),
                  bool(np.any(g("brv"))), bool(np.any(g("bout"))))
    brow = brow.astype(BF_NP)

    ident = np.eye(128, dtype=BF_NP)

    shared = dict(wP=wP, wI1=wI1, wGT=wGT, wGI=wGI, wRV=wRV, wOM=wOM,
                  vbc=vbc, colsd=cols, identd=ident, scal=scal, brow=brow)

    txt = g("txt_hidden").astype(BF_NP)
    vis = g("vis_hidden").astype(BF_NP)
    in_maps = []
    for c in range(B):
        m = dict(shared)
        m["txt"] = np.ascontiguousarray(txt[c])
        m["vis"] = np.ascontiguousarray(vis[c])
        in_maps.append(m)
    return in_maps, bias_flags


def kernel(**inputs):
    global LAST
    from concourse import bass_utils

    in_maps, bias_flags = _inputs_pack(inputs)
    key = ("v4", bias_flags)
    nc = _CACHE.get(key)
    if nc is None:
        nc = _build(bias_flags)
        _CACHE[key] = nc

    res = bass_utils.run_bass_kernel_spmd(
        nc, in_maps, core_ids=list(range(B)),
        trace=bool(os.environ.get("KERNEL_TRACE")),
    )
    LAST = res
    out = np.stack([np.asarray(res.results[c]["out"]) for c in range(B)], axis=0)
    return out.astype(np.float32)
